# revision 36
# baseline (speedup 1.0000x reference)
"""BERT self-attention (B=4, S=1024, D=1024, H=16) on 8 TRN2 NeuronCores.

Sharding: tensor-parallel over heads. Core c owns output dims
[c*128, (c+1)*128) of Wq/Wk/Wv (= heads 2c and 2c+1) and computes those
heads' attention for all 4 batches. seq is replicated (each core needs
all tokens). The host pre-transposes seq -> seqT [D, B*S] and the weight
shards -> [D, 128] (both cast to fp16); all matmuls run fp16 with fp32
PSUM accumulation. fp8 was measured and rejected: the softmax here is
sharp (scores reach +-9 sigma), so fp8's ~3-6% relative error on v or
exp lands nearly full-scale on the output (2-4e-2 rel err).

v3 vs the original kernel (173.7us -> 159.5us measured same-session):
 - (b, head)-granular pipeline: scores+exp for (b,h) are ACT-paced while
   the PE chews p@v of the previous half-slot and the next batch's QKV.
   The final slot inlines its own p@v at the exp tiles' dependency
   points, so only the m0=6 pair plus the division chain trails the last
   ACT (the tail division's copies/casts run on the then-idle scalar
   engine to shorten the DVE serial chain).
 - 12 dummy warmup matmuls on a scratch tile eat the PE HAM clock-gate
   ramp (1.2 -> 2.4 GHz after ~3.4us of activity) during the initial
   seq DMA wait, which is itself split across both hwdge queues (sync +
   scalar) since ACT is idle until the first exp.
 - softmax division: reciprocal_approx_fast off an SBUF-staged den row
   (the custom DVE op reads garbage from PSUM on HW - sim won't catch
   it), fp16 cast, K=1 broadcast matmul, fp16 bc staging (the DVE can
   read only ONE PSUM operand per instruction), TT multiply to fp16 out.
 - fp16 output DMA (half the out traffic; fp16 quantization of the
   output is ~1e-4 relative).

The softmax skips the max-subtraction: exp(s/8) <= e^10 fits fp16.

Measured dead ends (don't redo): DMA-xbar transposes for v cost 1.2us
per 128x128 tile and serialize the sync queue (+24us); K=64 row-pair
and M<=64 col-pair matmul co-issue is real on quiet queues (113/126
ns/MM microbenched) but the scores pairs are dep-gated by alternating
ACT completions through the 2-buffer PSUM rotation, so in-kernel
pairing bought nothing; PSUM's 8 banks are exactly consumed (scores
2x2 + qkv/bc/transpose 2 + pv 2), which blocks the den-matmul scheme
that would let p@v drop its ones column and col-pair.
"""

import numpy as np
from contextlib import ExitStack

import concourse.bass as bass
import concourse.tile as tile
from concourse import bacc, mybir
from concourse.bass_utils import run_bass_kernel_spmd

N_CORES = 8
B, S, D = 4, 1024, 1024
DPC = 128  # output dims per core (2 heads x 64)
HPC = 2  # heads per core
DV = 64  # head dim
KT = D // 128  # contraction tiles
NCH = S // 512  # 512-wide free-dim chunks per batch
VAUW = 130  # per-t8 vau row: [v_h0(64) | 1 | v_h1(64) | 1]
F32 = mybir.dt.float32
F16 = mybir.dt.float16
EXP = mybir.ActivationFunctionType.Exp
MULT = mybir.AluOpType.mult

# test.py may flip these to profile; the grading path leaves them alone.
TRACE = False
TRACE_KWARGS = {}
LAST_RESULTS = None

_CACHE = {}


def _emit(ctx, tc, seqT, wT, bias, ident, outcT):
    nc = tc.nc

    singles = ctx.enter_context(tc.tile_pool(name="singles", bufs=1))
    seq_pool = ctx.enter_context(tc.tile_pool(name="seq", bufs=2))
    qkv_pool = ctx.enter_context(tc.tile_pool(name="qkv", bufs=2))
    exp_pool = ctx.enter_context(tc.tile_pool(name="expT", bufs=34))
    small_pool = ctx.enter_context(tc.tile_pool(name="small", bufs=4))
    out_pool = ctx.enter_context(tc.tile_pool(name="out", bufs=4))
    psum_mm = ctx.enter_context(tc.tile_pool(name="psum_mm", bufs=2, space="PSUM"))
    psum_sc = ctx.enter_context(tc.tile_pool(name="psum_sc", bufs=2, space="PSUM"))
    psum_pv = ctx.enter_context(tc.tile_pool(name="psum_pv", bufs=2, space="PSUM"))

    w_sb = {}
    b_sb = {}

    def load_w(name):
        # one DMA per weight: DRAM [D, 128] -> SBUF [128, KT, 128]
        wt = singles.tile([128, KT, 128], F16, tag=f"w{name}", name=f"w{name}_sb")
        nc.sync.dma_start(wt[:], wT[name].rearrange("(k p) m -> p k m", p=128))
        w_sb[name] = wt
        bt = singles.tile([128, 1], F32, tag=f"b{name}", name=f"b{name}_sb")
        nc.gpsimd.dma_start(bt[:], bias[name][:])
        b_sb[name] = bt

    load_w("q")
    id_sb = singles.tile([128, 128], F16, tag="ident", name="id_sb")
    nc.gpsimd.dma_start(id_sb[:], ident[:])
    ones_sb = singles.tile([1, DV], F16, tag="ones", name="ones_sb")
    nc.gpsimd.memset(ones_sb[:], 1.0)

    # Persistent v tiles: [128 tok, t8, VAUW]; per t8 row is
    # [v_h0(64) | 1 | v_h1(64) | 1]. Three rotating sets.
    va_sets = []
    for sidx in range(3):
        va = singles.tile([128, KT, VAUW], F16, tag=f"vaug_{sidx}",
                          name=f"vaug_{sidx}")
        for t8 in range(KT):
            nc.gpsimd.memset(va[:, t8, DV : DV + 1], 1.0)
            nc.gpsimd.memset(va[:, t8, 2 * DV + 1 : 2 * DV + 2], 1.0)
        va_sets.append(va)

    all_exp = {}
    qkvT_by_b = {}

    def alloc_seq(b):
        # 4 sub-tiles of 2 k-tiles each so the first QKV matmuls only wait
        # on a quarter of the batch's tokens
        return [
            seq_pool.tile([128, 2, S], F16, tag=f"seqT{j}", name=f"seqT_b{b}p{j}")
            for j in range(4)
        ]

    def emit_dma_part(b, sq, j, eng=None):
        (eng or nc.sync).dma_start(
            sq[j][:],
            seqT[:, b * S : (b + 1) * S].rearrange("(k p) s -> p k s", p=128)[
                :, 2 * j : 2 * j + 2, :
            ],
        )

    def proj_units(b, sq, names):
        """Projection matmuls for batch b (kk-pair-major so each weight pair
        is reused for both 512-chunks before switching)."""
        units = []
        qkvT_by_b.setdefault(b, {})
        for name in names:
            dst = qkv_pool.tile([128, S], F16, tag=f"{name}T", name=f"{name}T_b{b}")
            qkvT_by_b[b][name] = dst
            pss = [
                psum_mm.tile([128, 512], F32, tag="mm", name=f"ps_{name}{b}{ic}")
                for ic in range(NCH)
            ]

            def mm2(name, kk0, ic, ps):
                for kk in (kk0, kk0 + 1):
                    nc.tensor.matmul(
                        ps[:],
                        w_sb[name][:, kk, :],
                        sq[kk // 2][:, kk % 2, ic * 512 : (ic + 1) * 512],
                        start=(kk == 0),
                        stop=(kk == KT - 1),
                    )

            for kk0 in range(0, KT, 2):
                for ic in range(NCH):
                    units.append(
                        (lambda name=name, kk0=kk0, ic=ic, ps=pss[ic]: mm2(
                            name, kk0, ic, ps
                        ), 432)
                    )

            def bias_add(name, ic, ps, dst):
                nc.vector.tensor_scalar_add(
                    dst[:, ic * 512 : (ic + 1) * 512], ps[:], b_sb[name][:]
                )

            for ic in range(NCH):
                units.append(
                    (lambda name=name, ic=ic, ps=pss[ic], dst=dst: bias_add(
                        name, ic, ps, dst
                    ), 0)
                )
        return units

    def vtr_units(b):
        """v token-major via PE transpose + one DVE copy per block into the
        [v_h0|1|v_h1|1] stationary tiles."""
        units = []
        va = va_sets[b % 3]
        for t8 in range(KT):

            def tr(t8=t8, va=va):
                vT = qkvT_by_b[b]["v"]
                pt = psum_mm.tile([128, 128], F16, tag="mm", name=f"vtr_{b}{t8}")
                nc.tensor.transpose(pt[:], vT[:, t8 * 128 : (t8 + 1) * 128], id_sb[:])
                dst = va[:, t8, 0 : 2 * (DV + 1)].rearrange(
                    "p (h x) -> p h x", h=2
                )[:, :, 0:DV]
                nc.vector.tensor_copy(
                    dst, pt[:].rearrange("p (h d) -> p h d", h=2)
                )

            units.append((tr, 280))
        return units

    def pv_units(b, h, tail=False):
        """p@v for (b, h) + softmax division. Units are m0-major (both ics of
        a t8-pair adjacent) so the tail slot can interleave them right after
        the exp tiles they need. With tail=True the division's copies/casts
        run on the (then idle) scalar engine to shorten the DVE serial chain.
        The division's PE matmul is deferred so the DVE reciprocal never
        gates the PE stream."""
        if tail:
            cp = nc.scalar.copy
        else:
            cp = nc.vector.tensor_copy
        units = []
        deferred = []
        va = va_sets[b % 3]
        rc32 = small_pool.tile([1, S], F32, tag="rc32", name=f"rc32_{b}{h}")
        rc16 = small_pool.tile([1, S], F16, tag="rc16", name=f"rc16_{b}{h}")
        of = out_pool.tile([DV, S], F16, tag="of", name=f"of_{b}{h}")
        pvs = [
            psum_pv.tile([DV + 1, 512], F32, tag="pv", name=f"pv_{b}{h}{ic}")
            for ic in range(NCH)
        ]

        def mm2(pv, ic, t80):
            # exp tile (t8, ic) holds [h0-chunk | h1-chunk]
            ex = all_exp[b]
            for t8 in (t80, t80 + 1):
                nc.tensor.matmul(
                    pv[:],
                    va[:, t8, h * (DV + 1) : (h + 1) * (DV + 1)],
                    ex[t8][ic][:, h * 512 : (h + 1) * 512],
                    start=(t8 == 0),
                    stop=(t8 == KT - 1),
                )

        for t80 in range(0, KT, 2):
            for ic in range(NCH):
                units.append(
                    (lambda pv=pvs[ic], ic=ic, t80=t80: mm2(pv, ic, t80), 432)
                )

        def recip(pv, ic):
            # custom-DVE reciprocal reads SBUF only; stage the den row
            den = small_pool.tile([1, 512], F32, tag="den", name=f"den_{b}{h}{ic}")
            cp(den[:], pv[DV : DV + 1, :])
            nc.vector.reciprocal_approx_fast(
                rc32[:, ic * 512 : (ic + 1) * 512], den[:]
            )
            cp(
                rc16[:, ic * 512 : (ic + 1) * 512],
                rc32[:, ic * 512 : (ic + 1) * 512],
            )

        for ic in range(NCH):
            units.append((lambda pv=pvs[ic], ic=ic: recip(pv, ic), 0))

        for ic in range(NCH):

            def div_unit(pv=pvs[ic], ic=ic):
                # K=1 matmul broadcasts 1/den over the 64 head dims; the DVE
                # can only read one PSUM operand, so stage bc in SBUF (fp16).
                bc = psum_mm.tile([DV, 512], F32, tag="mm", name=f"bc_{b}{h}{ic}")
                nc.tensor.matmul(
                    bc[:],
                    ones_sb[:],
                    rc16[:, ic * 512 : (ic + 1) * 512],
                    start=True,
                    stop=True,
                )
                bc_sb = small_pool.tile(
                    [DV, 512], F16, tag="bcs", name=f"bcs_{b}{h}{ic}"
                )
                cp(bc_sb[:], bc[:])
                nc.vector.tensor_tensor(
                    of[:, ic * 512 : (ic + 1) * 512], pv[0:DV, :], bc_sb[:], MULT
                )

            deferred.append((div_unit, 220))

        def dma_out():
            nc.sync.dma_start(
                outcT[h * DV : (h + 1) * DV, b * S : (b + 1) * S], of[:]
            )

        return units, deferred + [(dma_out, 0)]

    def emit_scores(b, filler, last=False):
        """Scores+exp for batch b: 16 (t8, ic) tiles; each [128,1024] tile
        holds [h0-chunk | h1-chunk] and is written by an adjacent pair of
        K=64 matmuls on disjoint PE row halves. Because BOTH pair members
        are released by the same tile-free event (one ACT drains the whole
        tile), they co-issue (~113 ns/MM measured vs 216 serial). ACT-paced,
        filler threaded between tiles. With last=True this slot's own p@v
        is inlined at its exp dependency points."""
        fq = list(filler)
        fi = 0
        cum = 0
        total = sum(c for _, c in fq) or 1
        qT = qkvT_by_b[b]["q"]
        kT = qkvT_by_b[b]["k"]
        ex = []
        all_exp[b] = ex
        if last:
            u0, d0 = pv_units(b, 0, tail=True)
            u1, d1 = pv_units(b, 1, tail=True)
        for t8 in range(KT):
            row = []
            for ic in range(NCH):
                ps = psum_sc.tile(
                    [128, S], F32, tag="sc2", name=f"sc_{b}{t8}{ic}"
                )
                for h in range(HPC):
                    hs = slice(h * DV, (h + 1) * DV)
                    nc.tensor.matmul(
                        ps[:, h * 512 : (h + 1) * 512],
                        kT[hs, t8 * 128 : (t8 + 1) * 128],
                        qT[hs, ic * 512 : (ic + 1) * 512],
                        start=True,
                        stop=True,
                    )
                et = exp_pool.tile(
                    [128, S], F16, tag="expT", name=f"ex_{b}{t8}{ic}"
                )
                nc.scalar.activation(et[:], ps[:], EXP, scale=0.125)
                row.append(et)
                pt = 2 * t8 + ic + 1  # 1..16 pacing points
                if last:
                    # filler (which frees the pv PSUM buffers this slot's
                    # own p@v needs) within the first 3 t8 groups; inline
                    # p@v afterwards so buffer waits never head-block
                    # still-queued PE work
                    want = (pt * total) // 6 if t8 < 3 else total
                else:
                    want = (pt * total) // (2 * KT)
                while fi < len(fq) and cum < want:
                    fn, c = fq[fi]
                    fn()
                    cum += c
                    fi += 1
            ex.append(row)
            if last and t8 >= 3 and t8 % 2 == 1:
                m0 = t8 - 3
                u0[m0][0]()
                u0[m0 + 1][0]()
                u1[m0][0]()
                u1[m0 + 1][0]()
        while fi < len(fq):
            fq[fi][0]()
            fi += 1
        if last:
            for x, _ in u0[KT - 2 :] + u1[KT - 2 :] + d0 + d1:
                x()

    # ---- pipeline ----
    # HAM warmup: the PE clock-gate defaults to 1.2 GHz and only reaches
    # 2.4 GHz after ~3.4us of sustained activity. Burn dummy matmuls on a
    # scratch tile during the initial DMA wait so the first real matmuls
    # run warm.
    scratch = singles.tile([128, 512], F16, tag="scr", name="scratch_sb")
    nc.vector.memset(scratch[:], 0.0)
    for i in range(12):
        wps = psum_mm.tile([128, 512], F32, tag="mm", name=f"warm{i}")
        nc.tensor.matmul(
            wps[:], scratch[:, 0:128], scratch[:], start=True, stop=True
        )

    # Prologue: batch 0's seq DMA split over both hwdge queues (the scalar
    # engine is idle until the first exp); V(0) and its transposes ride as
    # filler inside scores(0, h0).
    sq = alloc_seq(0)
    emit_dma_part(0, sq, 0, eng=nc.scalar)
    load_w("k")
    load_w("v")
    emit_dma_part(0, sq, 1, eng=nc.scalar)
    for j in range(2, 4):
        emit_dma_part(0, sq, j)
    for u, _ in proj_units(0, sq, ("q", "k")):
        u()
    vwork0 = proj_units(0, sq, ("v",)) + vtr_units(0)

    for b in range(B):
        filler = []
        if b > 0:
            u0, d0 = pv_units(b - 1, 0)
            u1, d1 = pv_units(b - 1, 1)
            filler += u0 + u1 + d0 + d1
        else:
            filler += vwork0
        if b + 1 < B:
            sq_next = alloc_seq(b + 1)
            for j in range(4):
                emit_dma_part(b + 1, sq_next, j)
            filler += proj_units(b + 1, sq_next, ("q", "k", "v"))
            filler += vtr_units(b + 1)
        emit_scores(b, filler, last=(b == B - 1))



def _build():
    if "nc" in _CACHE:
        return _CACHE["nc"]
    nc = bacc.Bacc(
        "TRN2",
        target_bir_lowering=False,
        debug=False,
        enable_asserts=False,
        num_devices=N_CORES,
    )
    seqT = nc.dram_tensor("seqT", [D, B * S], F16, kind="ExternalInput").ap()
    wT = {
        name: nc.dram_tensor(f"w{name}T", [D, DPC], F16, kind="ExternalInput").ap()
        for name in ("q", "k", "v")
    }
    bias = {
        name: nc.dram_tensor(f"b{name}", [DPC, 1], F32, kind="ExternalInput").ap()
        for name in ("q", "k", "v")
    }
    ident = nc.dram_tensor("ident", [128, 128], F16, kind="ExternalInput").ap()
    outcT = nc.dram_tensor("outcT", [HPC * DV, B * S], F16, kind="ExternalOutput").ap()

    with tile.TileContext(nc) as tc:
        with ExitStack() as ctx:
            _emit(ctx, tc, seqT, wT, bias, ident, outcT)
    nc.compile()
    _CACHE["nc"] = nc
    return nc


def make_in_maps(seq, Wq, bq, Wk, bk, Wv, bv):
    f16 = np.float16
    seq = np.asarray(seq, np.float32)
    seqT_full = np.ascontiguousarray(seq.reshape(B * S, D).T.astype(f16))
    in_maps = []
    for c in range(N_CORES):
        sl = slice(c * DPC, (c + 1) * DPC)
        in_maps.append(
            {
                "seqT": seqT_full,
                "wqT": np.ascontiguousarray(np.asarray(Wq, np.float32)[sl].T.astype(f16)),
                "wkT": np.ascontiguousarray(np.asarray(Wk, np.float32)[sl].T.astype(f16)),
                "wvT": np.ascontiguousarray(np.asarray(Wv, np.float32)[sl].T.astype(f16)),
                "bq": np.ascontiguousarray(np.asarray(bq, np.float32)[sl].reshape(DPC, 1)),
                "bk": np.ascontiguousarray(np.asarray(bk, np.float32)[sl].reshape(DPC, 1)),
                "bv": np.ascontiguousarray(np.asarray(bv, np.float32)[sl].reshape(DPC, 1)),
                "ident": np.eye(128, dtype=f16),
            }
        )
    return in_maps


def assemble(results):
    """[cores][h*64+d, b*1024+i] -> [B, S, D]"""
    out = np.empty((B, S, D), np.float32)
    for c in range(N_CORES):
        r = np.asarray(results[c]["outcT"], np.float32).reshape(DPC, B, S)
        out[:, :, c * DPC : (c + 1) * DPC] = r.transpose(1, 2, 0)
    return out


def kernel(seq, Wq, bq, Wk, bk, Wv, bv):
    global LAST_RESULTS
    nc = _build()
    in_maps = make_in_maps(seq, Wq, bq, Wk, bk, Wv, bv)
    res = run_bass_kernel_spmd(
        nc, in_maps, core_ids=list(range(N_CORES)), trace=TRACE, **TRACE_KWARGS
    )
    LAST_RESULTS = res
    return assemble(res.results)


# revision 38
# speedup vs baseline: 1.0162x; 1.0162x over previous
"""BERT self-attention (B=4, S=1024, D=1024, H=16) on 8 TRN2 NeuronCores.

Sharding: tensor-parallel over heads. Core c owns output dims
[c*128, (c+1)*128) of Wq/Wk/Wv (= heads 2c and 2c+1) and computes those
heads' attention for all 4 batches. seq is replicated (each core needs
all tokens). The host pre-transposes seq -> seqT [D, B*S] and the weight
shards -> [D, 128] (both cast to fp16); all matmuls run fp16 with fp32
PSUM accumulation. fp8 was measured and rejected: the softmax here is
sharp (scores reach +-9 sigma), so fp8's ~3-6% relative error on v or
exp lands nearly full-scale on the output (2-4e-2 rel err).

v3 vs the original kernel (173.7us -> 159.5us measured same-session):
 - (b, head)-granular pipeline: scores+exp for (b,h) are ACT-paced while
   the PE chews p@v of the previous half-slot and the next batch's QKV.
   The final slot inlines its own p@v at the exp tiles' dependency
   points, so only the m0=6 pair plus the division chain trails the last
   ACT (the tail division's copies/casts run on the then-idle scalar
   engine to shorten the DVE serial chain).
 - 12 dummy warmup matmuls on a scratch tile eat the PE HAM clock-gate
   ramp (1.2 -> 2.4 GHz after ~3.4us of activity) during the initial
   seq DMA wait, which is itself split across both hwdge queues (sync +
   scalar) since ACT is idle until the first exp.
 - softmax division: reciprocal_approx_fast off an SBUF-staged den row
   (the custom DVE op reads garbage from PSUM on HW - sim won't catch
   it), fp16 cast, K=1 broadcast matmul, fp16 bc staging (the DVE can
   read only ONE PSUM operand per instruction), TT multiply to fp16 out.
 - fp16 output DMA (half the out traffic; fp16 quantization of the
   output is ~1e-4 relative).

The softmax skips the max-subtraction: exp(s/8) <= e^10 fits fp16.

Measured dead ends (don't redo): DMA-xbar transposes for v cost 1.2us
per 128x128 tile and serialize the sync queue (+24us); K=64 row-pair
and M<=64 col-pair matmul co-issue is real on quiet queues (113/126
ns/MM microbenched) but the scores pairs are dep-gated by alternating
ACT completions through the 2-buffer PSUM rotation, so in-kernel
pairing bought nothing; PSUM's 8 banks are exactly consumed (scores
2x2 + qkv/bc/transpose 2 + pv 2), which blocks the den-matmul scheme
that would let p@v drop its ones column and col-pair.
"""

import numpy as np
from contextlib import ExitStack

import concourse.bass as bass
import concourse.tile as tile
from concourse import bacc, mybir
from concourse.bass_utils import run_bass_kernel_spmd

N_CORES = 8
B, S, D = 4, 1024, 1024
DPC = 128  # output dims per core (2 heads x 64)
HPC = 2  # heads per core
DV = 64  # head dim
KT = D // 128  # contraction tiles
NCH = S // 512  # 512-wide free-dim chunks per batch
VAUW = 130  # per-t8 vau row: [v_h0(64) | 1 | v_h1(64) | 1]
F32 = mybir.dt.float32
F16 = mybir.dt.float16
EXP = mybir.ActivationFunctionType.Exp
MULT = mybir.AluOpType.mult

# test.py may flip these to profile; the grading path leaves them alone.
TRACE = False
TRACE_KWARGS = {}
LAST_RESULTS = None

_CACHE = {}


def _emit(ctx, tc, seqT, wT, bias, ident, outcT):
    nc = tc.nc

    singles = ctx.enter_context(tc.tile_pool(name="singles", bufs=1))
    seq_pool = ctx.enter_context(tc.tile_pool(name="seq", bufs=2))
    qkv_pool = ctx.enter_context(tc.tile_pool(name="qkv", bufs=2))
    exp_pool = ctx.enter_context(tc.tile_pool(name="expT", bufs=34))
    small_pool = ctx.enter_context(tc.tile_pool(name="small", bufs=4))
    out_pool = ctx.enter_context(tc.tile_pool(name="out", bufs=4))
    psum_mm = ctx.enter_context(tc.tile_pool(name="psum_mm", bufs=2, space="PSUM"))
    psum_sc = ctx.enter_context(tc.tile_pool(name="psum_sc", bufs=2, space="PSUM"))
    psum_pv = ctx.enter_context(tc.tile_pool(name="psum_pv", bufs=2, space="PSUM"))

    w_sb = {}
    b_sb = {}

    def load_w(name):
        # one DMA per weight: DRAM [D, 128] -> SBUF [128, KT, 128]
        wt = singles.tile([128, KT, 128], F16, tag=f"w{name}", name=f"w{name}_sb")
        nc.sync.dma_start(wt[:], wT[name].rearrange("(k p) m -> p k m", p=128))
        w_sb[name] = wt
        bt = singles.tile([128, 1], F32, tag=f"b{name}", name=f"b{name}_sb")
        nc.gpsimd.dma_start(bt[:], bias[name][:])
        b_sb[name] = bt

    load_w("q")
    id_sb = singles.tile([128, 128], F16, tag="ident", name="id_sb")
    nc.gpsimd.dma_start(id_sb[:], ident[:])
    ones_sb = singles.tile([1, DV], F16, tag="ones", name="ones_sb")
    nc.gpsimd.memset(ones_sb[:], 1.0)

    # Persistent v tiles: [128 tok, t8, VAUW]; per t8 row is
    # [v_h0(64) | 1 | v_h1(64) | 1]. Three rotating sets.
    va_sets = []
    for sidx in range(3):
        va = singles.tile([128, KT, VAUW], F16, tag=f"vaug_{sidx}",
                          name=f"vaug_{sidx}")
        for t8 in range(KT):
            nc.gpsimd.memset(va[:, t8, DV : DV + 1], 1.0)
            nc.gpsimd.memset(va[:, t8, 2 * DV + 1 : 2 * DV + 2], 1.0)
        va_sets.append(va)

    all_exp = {}
    qkvT_by_b = {}

    def alloc_seq(b):
        # 4 sub-tiles of 2 k-tiles each so the first QKV matmuls only wait
        # on a quarter of the batch's tokens
        return [
            seq_pool.tile([128, 2, S], F16, tag=f"seqT{j}", name=f"seqT_b{b}p{j}")
            for j in range(4)
        ]

    def emit_dma_part(b, sq, j, eng=None):
        (eng or nc.sync).dma_start(
            sq[j][:],
            seqT[:, b * S : (b + 1) * S].rearrange("(k p) s -> p k s", p=128)[
                :, 2 * j : 2 * j + 2, :
            ],
        )

    def proj_units(b, sq, names):
        """Projection matmuls for batch b (kk-pair-major so each weight pair
        is reused for both 512-chunks before switching)."""
        units = []
        qkvT_by_b.setdefault(b, {})
        for name in names:
            dst = qkv_pool.tile([128, S], F16, tag=f"{name}T", name=f"{name}T_b{b}")
            qkvT_by_b[b][name] = dst
            pss = [
                psum_mm.tile([128, 512], F32, tag="mm", name=f"ps_{name}{b}{ic}")
                for ic in range(NCH)
            ]

            def mm2(name, kk0, ic, ps):
                for kk in (kk0, kk0 + 1):
                    nc.tensor.matmul(
                        ps[:],
                        w_sb[name][:, kk, :],
                        sq[kk // 2][:, kk % 2, ic * 512 : (ic + 1) * 512],
                        start=(kk == 0),
                        stop=(kk == KT - 1),
                    )

            for kk0 in range(0, KT, 2):
                for ic in range(NCH):
                    units.append(
                        (lambda name=name, kk0=kk0, ic=ic, ps=pss[ic]: mm2(
                            name, kk0, ic, ps
                        ), 432)
                    )

            def bias_add(name, ic, ps, dst):
                nc.vector.tensor_scalar_add(
                    dst[:, ic * 512 : (ic + 1) * 512], ps[:], b_sb[name][:]
                )

            for ic in range(NCH):
                units.append(
                    (lambda name=name, ic=ic, ps=pss[ic], dst=dst: bias_add(
                        name, ic, ps, dst
                    ), 0)
                )
        return units

    def vtr_units(b):
        """v token-major via PE transpose + one DVE copy per block into the
        [v_h0|1|v_h1|1] stationary tiles."""
        units = []
        va = va_sets[b % 3]
        for t8 in range(KT):

            def tr(t8=t8, va=va):
                vT = qkvT_by_b[b]["v"]
                pt = psum_mm.tile([128, 128], F16, tag="mm", name=f"vtr_{b}{t8}")
                nc.tensor.transpose(pt[:], vT[:, t8 * 128 : (t8 + 1) * 128], id_sb[:])
                dst = va[:, t8, 0 : 2 * (DV + 1)].rearrange(
                    "p (h x) -> p h x", h=2
                )[:, :, 0:DV]
                nc.vector.tensor_copy(
                    dst, pt[:].rearrange("p (h d) -> p h d", h=2)
                )

            units.append((tr, 280))
        return units

    def pv_units(b, h, tail=False):
        """p@v for (b, h) + softmax division. Units are m0-major (both ics of
        a t8-pair adjacent) so the tail slot can interleave them right after
        the exp tiles they need. With tail=True the division's copies/casts
        run on the (then idle) scalar engine to shorten the DVE serial chain.
        The division's PE matmul is deferred so the DVE reciprocal never
        gates the PE stream."""
        if tail:
            cp = nc.scalar.copy
        else:
            cp = nc.vector.tensor_copy
        units = []
        deferred = []
        va = va_sets[b % 3]
        rc32 = small_pool.tile([1, S], F32, tag="rc32", name=f"rc32_{b}{h}")
        rc16 = small_pool.tile([1, S], F16, tag="rc16", name=f"rc16_{b}{h}")
        of = out_pool.tile([DV, S], F16, tag="of", name=f"of_{b}{h}")
        pvs = [
            psum_pv.tile([DV + 1, 512], F32, tag="pv", name=f"pv_{b}{h}{ic}")
            for ic in range(NCH)
        ]

        def mm2(pv, ic, t80):
            # exp tile (t8, ic) holds [h0-chunk | h1-chunk]
            ex = all_exp[b]
            for t8 in (t80, t80 + 1):
                nc.tensor.matmul(
                    pv[:],
                    va[:, t8, h * (DV + 1) : (h + 1) * (DV + 1)],
                    ex[t8][ic][:, h * 512 : (h + 1) * 512],
                    start=(t8 == 0),
                    stop=(t8 == KT - 1),
                )

        for t80 in range(0, KT, 2):
            for ic in range(NCH):
                units.append(
                    (lambda pv=pvs[ic], ic=ic, t80=t80: mm2(pv, ic, t80), 432)
                )

        def recip(pv, ic):
            # custom-DVE reciprocal reads SBUF only; stage the den row
            den = small_pool.tile([1, 512], F32, tag="den", name=f"den_{b}{h}{ic}")
            cp(den[:], pv[DV : DV + 1, :])
            nc.vector.reciprocal_approx_fast(
                rc32[:, ic * 512 : (ic + 1) * 512], den[:]
            )
            cp(
                rc16[:, ic * 512 : (ic + 1) * 512],
                rc32[:, ic * 512 : (ic + 1) * 512],
            )

        for ic in range(NCH):
            units.append((lambda pv=pvs[ic], ic=ic: recip(pv, ic), 0))

        for ic in range(NCH):

            def div_unit(pv=pvs[ic], ic=ic):
                # K=1 matmul broadcasts 1/den over the 64 head dims; the DVE
                # can only read one PSUM operand, so stage bc in SBUF (fp16).
                bc = psum_mm.tile([DV, 512], F32, tag="mm", name=f"bc_{b}{h}{ic}")
                nc.tensor.matmul(
                    bc[:],
                    ones_sb[:],
                    rc16[:, ic * 512 : (ic + 1) * 512],
                    start=True,
                    stop=True,
                )
                bc_sb = small_pool.tile(
                    [DV, 512], F16, tag="bcs", name=f"bcs_{b}{h}{ic}"
                )
                cp(bc_sb[:], bc[:])
                nc.vector.tensor_tensor(
                    of[:, ic * 512 : (ic + 1) * 512], pv[0:DV, :], bc_sb[:], MULT
                )

            deferred.append((div_unit, 220))

        def dma_out():
            nc.sync.dma_start(
                outcT[h * DV : (h + 1) * DV, b * S : (b + 1) * S], of[:]
            )

        return units, deferred + [(dma_out, 0)]

    def emit_scores(b, filler, last=False):
        """Scores+exp for batch b: 16 (t8, ic) tiles; each [128,1024] tile
        holds [h0-chunk | h1-chunk] and is written by an adjacent pair of
        K=64 matmuls on disjoint PE row halves. Because BOTH pair members
        are released by the same tile-free event (one ACT drains the whole
        tile), they co-issue (~113 ns/MM measured vs 216 serial). ACT-paced,
        filler threaded between tiles. With last=True this slot's own p@v
        is inlined at its exp dependency points."""
        fq = list(filler)
        fi = 0
        cum = 0
        total = sum(c for _, c in fq) or 1
        qT = qkvT_by_b[b]["q"]
        kT = qkvT_by_b[b]["k"]
        ex = []
        all_exp[b] = ex
        if last:
            u0, d0 = pv_units(b, 0, tail=True)
            u1, d1 = pv_units(b, 1, tail=True)
        for t8 in range(KT):
            row = []
            for ic in range(NCH):
                ps = psum_sc.tile(
                    [128, S], F32, tag="sc2", name=f"sc_{b}{t8}{ic}"
                )
                for h in range(HPC):
                    hs = slice(h * DV, (h + 1) * DV)
                    nc.tensor.matmul(
                        ps[:, h * 512 : (h + 1) * 512],
                        kT[hs, t8 * 128 : (t8 + 1) * 128],
                        qT[hs, ic * 512 : (ic + 1) * 512],
                        start=True,
                        stop=True,
                    )
                et = exp_pool.tile(
                    [128, S], F16, tag="expT", name=f"ex_{b}{t8}{ic}"
                )
                nc.scalar.activation(et[:], ps[:], EXP, scale=0.125)
                row.append(et)
                pt = 2 * t8 + ic + 1  # 1..16 pacing points
                if last:
                    # filler (which frees the pv PSUM buffers this slot's
                    # own p@v needs) within the first 3 t8 groups; inline
                    # p@v afterwards so buffer waits never head-block
                    # still-queued PE work
                    want = (pt * len(fq)) // 6 if t8 < 3 else len(fq)
                else:
                    want = (pt * len(fq)) // (2 * KT + 2)
                while fi < want:
                    fq[fi][0]()
                    fi += 1
            ex.append(row)
            if last and t8 >= 3 and t8 % 2 == 1:
                m0 = t8 - 3
                u0[m0][0]()
                u0[m0 + 1][0]()
                u1[m0][0]()
                u1[m0 + 1][0]()
        while fi < len(fq):
            fq[fi][0]()
            fi += 1
        if last:
            for x, _ in u0[KT - 2 :] + u1[KT - 2 :] + d0 + d1:
                x()

    # ---- pipeline ----
    # HAM warmup: the PE clock-gate defaults to 1.2 GHz and only reaches
    # 2.4 GHz after ~3.4us of sustained activity. Burn dummy matmuls on a
    # scratch tile during the initial DMA wait so the first real matmuls
    # run warm.
    scratch = singles.tile([128, 512], F16, tag="scr", name="scratch_sb")
    nc.vector.memset(scratch[:], 0.0)
    for i in range(18):
        wps = psum_mm.tile([128, 512], F32, tag="mm", name=f"warm{i}")
        nc.tensor.matmul(
            wps[:], scratch[:, 0:128], scratch[:], start=True, stop=True
        )

    # Prologue: batch 0's seq DMA split over both hwdge queues (the scalar
    # engine is idle until the first exp); V(0) and its transposes ride as
    # filler inside scores(0, h0).
    sq = alloc_seq(0)
    emit_dma_part(0, sq, 0, eng=nc.scalar)
    load_w("k")
    load_w("v")
    emit_dma_part(0, sq, 1, eng=nc.scalar)
    for j in range(2, 4):
        emit_dma_part(0, sq, j)
    for u, _ in proj_units(0, sq, ("q", "k")):
        u()
    vwork0 = proj_units(0, sq, ("v",)) + vtr_units(0)

    for b in range(B):
        filler = []
        if b > 0:
            u0, d0 = pv_units(b - 1, 0)
            u1, d1 = pv_units(b - 1, 1)
            filler += u0 + u1 + d0 + d1
        else:
            filler += vwork0
        if b + 1 < B:
            sq_next = alloc_seq(b + 1)
            for j in range(4):
                emit_dma_part(b + 1, sq_next, j)
            filler += proj_units(b + 1, sq_next, ("q", "k", "v"))
            filler += vtr_units(b + 1)
        emit_scores(b, filler, last=(b == B - 1))



def _build():
    if "nc" in _CACHE:
        return _CACHE["nc"]
    nc = bacc.Bacc(
        "TRN2",
        target_bir_lowering=False,
        debug=False,
        enable_asserts=False,
        num_devices=N_CORES,
    )
    seqT = nc.dram_tensor("seqT", [D, B * S], F16, kind="ExternalInput").ap()
    wT = {
        name: nc.dram_tensor(f"w{name}T", [D, DPC], F16, kind="ExternalInput").ap()
        for name in ("q", "k", "v")
    }
    bias = {
        name: nc.dram_tensor(f"b{name}", [DPC, 1], F32, kind="ExternalInput").ap()
        for name in ("q", "k", "v")
    }
    ident = nc.dram_tensor("ident", [128, 128], F16, kind="ExternalInput").ap()
    outcT = nc.dram_tensor("outcT", [HPC * DV, B * S], F16, kind="ExternalOutput").ap()

    with tile.TileContext(nc) as tc:
        with ExitStack() as ctx:
            _emit(ctx, tc, seqT, wT, bias, ident, outcT)
    nc.compile()
    _CACHE["nc"] = nc
    return nc


def make_in_maps(seq, Wq, bq, Wk, bk, Wv, bv):
    f16 = np.float16
    seq = np.asarray(seq, np.float32)
    seqT_full = np.ascontiguousarray(seq.reshape(B * S, D).T.astype(f16))
    in_maps = []
    for c in range(N_CORES):
        sl = slice(c * DPC, (c + 1) * DPC)
        in_maps.append(
            {
                "seqT": seqT_full,
                "wqT": np.ascontiguousarray(np.asarray(Wq, np.float32)[sl].T.astype(f16)),
                "wkT": np.ascontiguousarray(np.asarray(Wk, np.float32)[sl].T.astype(f16)),
                "wvT": np.ascontiguousarray(np.asarray(Wv, np.float32)[sl].T.astype(f16)),
                "bq": np.ascontiguousarray(np.asarray(bq, np.float32)[sl].reshape(DPC, 1)),
                "bk": np.ascontiguousarray(np.asarray(bk, np.float32)[sl].reshape(DPC, 1)),
                "bv": np.ascontiguousarray(np.asarray(bv, np.float32)[sl].reshape(DPC, 1)),
                "ident": np.eye(128, dtype=f16),
            }
        )
    return in_maps


def assemble(results):
    """[cores][h*64+d, b*1024+i] -> [B, S, D]"""
    out = np.empty((B, S, D), np.float32)
    for c in range(N_CORES):
        r = np.asarray(results[c]["outcT"], np.float32).reshape(DPC, B, S)
        out[:, :, c * DPC : (c + 1) * DPC] = r.transpose(1, 2, 0)
    return out


def kernel(seq, Wq, bq, Wk, bk, Wv, bv):
    global LAST_RESULTS
    nc = _build()
    in_maps = make_in_maps(seq, Wq, bq, Wk, bk, Wv, bv)
    res = run_bass_kernel_spmd(
        nc, in_maps, core_ids=list(range(N_CORES)), trace=TRACE, **TRACE_KWARGS
    )
    LAST_RESULTS = res
    return assemble(res.results)
